# revision 13
# baseline (speedup 1.0000x reference)
"""DualGATEncoder Trainium2 kernel (8 NeuronCores, SPMD).

Sharding: nodes 1250/core (contiguous); per-edge work owned by the
destination node's core; edges sorted by local dst into 128-dst blocks and
128-edge chunks (padded with inert slots).  Dense MLP/linear stages are
data-parallel over nodes and flow feature-major [C, n] through the
TensorEngine.  Per GAT layer each core computes xl/xr for its own nodes; one
AllGather builds the full xl table; the edge stage gathers xl[src]/xr[dst]
rows with batched dma_gather, computes GATv2 attention (softmax without
max-subtraction -- shift invariant, logits are O(1)) and segment-sums via
one-hot matmuls accumulating in PSUM.  Both modalities are fused so one
gather serves packet+time.  Compute dtype bf16, f32 PSUM accumulation.
"""
import sys
if "/opt/trn_rl_repo" not in sys.path:
    sys.path.insert(0, "/opt/trn_rl_repo")

import numpy as np
import ml_dtypes

import concourse.bass as bass
import concourse.bacc as bacc
import concourse.tile as tile
from concourse import mybir
from concourse.bass_utils import run_bass_kernel_spmd
from concourse.masks import make_identity

BF16 = ml_dtypes.bfloat16
f32 = mybir.dt.float32
bf16 = mybir.dt.bfloat16
i16 = mybir.dt.int16
i32 = mybir.dt.int32

NCORES = 8
N = 10000
NLOC = N // NCORES            # 1250
P = 128
NBLK = (NLOC + P - 1) // P    # 10 dst blocks; last has 98 real rows
NPAD = NBLK * P               # 1280
IN_DIM = 256
HID = 128
FEAT = 64 * 64                # 4096
GRP = 4                       # chunks per dma_gather group
D1 = 1024                     # merged gat1 width (2 modalities x 4 heads x 128)
D2 = 256                      # merged gat2 width (2 modalities x 128)

AluOp = mybir.AluOpType
ActFn = mybir.ActivationFunctionType


def _bf(x):
    return np.ascontiguousarray(np.asarray(x, dtype=np.float32).astype(BF16))


def _f32(x):
    return np.ascontiguousarray(np.asarray(x, dtype=np.float32))


# ---------------------------------------------------------------- host prep

def _prep_edges(edge_index):
    src = np.concatenate([np.asarray(edge_index[0]), np.arange(N, dtype=np.int64)])
    dst = np.concatenate([np.asarray(edge_index[1]), np.arange(N, dtype=np.int64)])
    src = src.astype(np.int32)
    dst = dst.astype(np.int32)

    per_core = []
    mhat = 0
    for r in range(NCORES):
        m = (dst >= r * NLOC) & (dst < (r + 1) * NLOC)
        s_r = src[m]
        d_r = dst[m] - r * NLOC
        order = np.argsort(d_r, kind="stable")
        s_r, d_r = s_r[order], d_r[order]
        blk = d_r // P
        bounds = np.searchsorted(blk, np.arange(NBLK + 1))
        per_core.append((s_r, d_r, bounds))
        for b in range(NBLK):
            nb = int(bounds[b + 1] - bounds[b])
            mhat = max(mhat, -(-nb // P))
    MG = -(-mhat // GRP) * GRP  # chunks per block, multiple of GRP

    srcs, dsts, cols = [], [], []
    for r in range(NCORES):
        s_r, d_r, bounds = per_core[r]
        sarr = np.zeros((NBLK, MG * P), np.int16)
        darr = np.zeros((NBLK, MG * P), np.int16)
        carr = np.full((NBLK, MG * P), -1.0, np.float32)
        for b in range(NBLK):
            lo, hi = int(bounds[b]), int(bounds[b + 1])
            nb = hi - lo
            sarr[b, :nb] = s_r[lo:hi]
            darr[b, :nb] = d_r[lo:hi]
            carr[b, :nb] = (d_r[lo:hi] - b * P).astype(np.float32)

        def pack(arr):
            # dma_gather idx packing: slot j -> [j%16, j//16] per group,
            # replicated over the 8 Q7-core partition windows.
            ngpb = MG // GRP
            a = arr.reshape(NBLK * ngpb, GRP * 8, 16)
            a = a.transpose(2, 0, 1).reshape(16, NBLK * ngpb * GRP * 8)
            return np.ascontiguousarray(np.tile(a, (8, 1)))
        srcs.append(pack(sarr))
        dsts.append(pack(darr))
        cc = carr.reshape(NBLK * MG, P).T  # [P slot, NBLK*MG chunk]
        cols.append(np.ascontiguousarray(cc.astype(BF16)))
    return srcs, dsts, cols, MG


def _prep_weights(params):
    g = {}
    p = params
    for m, key in (("p", "packet_ext"), ("t", "time_ext")):
        g[f"w1_{m}"] = _f32(p[key]["W1"])
        g[f"b1_{m}"] = _f32(p[key]["b1"]).reshape(HID, 1)
        g[f"w2_{m}"] = _bf(p[key]["W2"])
        g[f"b2_{m}"] = _f32(p[key]["b2"]).reshape(IN_DIM, 1)
    wl1 = np.concatenate([_f32(p["packet_gat1"]["Wl"]), _f32(p["time_gat1"]["Wl"])], 1)
    wr1 = np.concatenate([_f32(p["packet_gat1"]["Wr"]), _f32(p["time_gat1"]["Wr"])], 1)
    g["wl1"] = _bf(wl1)
    g["wr1"] = _bf(wr1)
    g["bl1"] = _bf(np.concatenate(
        [_f32(p["packet_gat1"]["bl"]), _f32(p["time_gat1"]["bl"])])).reshape(1, D1)
    g["br1"] = _bf(np.concatenate(
        [_f32(p["packet_gat1"]["br"]), _f32(p["time_gat1"]["br"])])).reshape(1, D1)
    g["gb1"] = _bf(np.concatenate(
        [_f32(p["packet_gat1"]["bias"]), _f32(p["time_gat1"]["bias"])])).reshape(1, D1)
    g["att1"] = _bf(np.concatenate(
        [_f32(p["packet_gat1"]["att"]), _f32(p["time_gat1"]["att"])], 0).T)  # [128,8]
    bl2, br2, gb2, att2 = [], [], [], []
    for m, key in (("p", "packet_gat2"), ("t", "time_gat2")):
        wl2 = _f32(p[key]["Wl"])
        wr2 = _f32(p[key]["Wr"])
        g[f"wl2_{m}"] = _bf(wl2)
        g[f"wr2_{m}"] = _bf(wr2)
        bl2.append(_f32(p[key]["bl"]) - wl2.sum(0))   # elu(-1) fold
        br2.append(_f32(p[key]["br"]) - wr2.sum(0))
        gb2.append(_f32(p[key]["bias"]))
        att2.append(_f32(p[key]["att"]))
    g["bl2"] = _bf(np.concatenate(bl2)).reshape(1, D2)
    g["br2"] = _bf(np.concatenate(br2)).reshape(1, D2)
    g["gb2"] = _bf(np.concatenate(gb2)).reshape(1, D2)
    g["att2"] = _bf(np.concatenate(att2, 0).T)        # [128, 2]
    w1e = _f32(p["enh"]["W1"])
    g["w1e"] = _f32(w1e)
    g["b1e"] = np.ascontiguousarray(
        (_f32(p["enh"]["b1"]) - w1e.sum(0)).reshape(4, P).T)  # [128, 4]
    g["w2e"] = _f32(p["enh"]["W2"])
    g["b2e"] = _f32(p["enh"]["b2"]).reshape(IN_DIM, 1)
    return g


# ---------------------------------------------------------------- device build

def _rh(ap, nh):
    return ap.rearrange("p (h c) -> p h c", h=nh)


def _build(MG, edge_blocks=NBLK):
    NGPB = MG // GRP
    NG = NBLK * NGPB
    nc = bacc.Bacc("TRN2", target_bir_lowering=False, debug=False,
                   num_devices=NCORES)

    featT = {m: nc.dram_tensor(f"featT_{m}", [FEAT, NLOC], f32, kind="ExternalInput")
             for m in ("p", "t")}
    src16 = nc.dram_tensor("src16", [P, NG * GRP * 8], i16, kind="ExternalInput")
    dst16 = nc.dram_tensor("dst16", [P, NG * GRP * 8], i16, kind="ExternalInput")
    dcol_in = nc.dram_tensor("dcol", [P, NBLK * MG], bf16, kind="ExternalInput")
    W = {}
    for m in ("p", "t"):
        W[f"w1_{m}"] = nc.dram_tensor(f"w1_{m}", [FEAT, HID], f32, kind="ExternalInput")
        W[f"b1_{m}"] = nc.dram_tensor(f"b1_{m}", [HID, 1], f32, kind="ExternalInput")
        W[f"w2_{m}"] = nc.dram_tensor(f"w2_{m}", [HID, IN_DIM], bf16, kind="ExternalInput")
        W[f"b2_{m}"] = nc.dram_tensor(f"b2_{m}", [IN_DIM, 1], f32, kind="ExternalInput")
        W[f"wl2_{m}"] = nc.dram_tensor(f"wl2_{m}", [4 * HID, HID], bf16, kind="ExternalInput")
        W[f"wr2_{m}"] = nc.dram_tensor(f"wr2_{m}", [4 * HID, HID], bf16, kind="ExternalInput")
    for nm, shp, dt in (
        ("wl1", [IN_DIM, D1], bf16), ("wr1", [IN_DIM, D1], bf16),
        ("bl1", [1, D1], bf16), ("br1", [1, D1], bf16), ("gb1", [1, D1], bf16),
        ("att1", [P, 8], bf16),
        ("bl2", [1, D2], bf16), ("br2", [1, D2], bf16), ("gb2", [1, D2], bf16),
        ("att2", [P, 2], bf16),
        ("w1e", [IN_DIM, 4 * HID], f32), ("b1e", [P, 4], f32),
        ("w2e", [4 * HID, IN_DIM], f32), ("b2e", [IN_DIM, 1], f32),
    ):
        W[nm] = nc.dram_tensor(nm, shp, dt, kind="ExternalInput")
    outT = nc.dram_tensor("outT", [IN_DIM, NLOC], f32, kind="ExternalOutput")

    ag1in = nc.dram_tensor("ag1in", [NLOC, D1], bf16)
    XL1 = nc.dram_tensor("XL1", [N, D1], bf16, addr_space="Shared")
    XR1 = nc.dram_tensor("XR1", [NLOC, D1], bf16)
    ag2in = nc.dram_tensor("ag2in", [NLOC, D2], bf16)
    XL2 = nc.dram_tensor("XL2", [N, D2], bf16, addr_space="Shared")
    XR2 = nc.dram_tensor("XR2", [NLOC, D2], bf16)

    with tile.TileContext(nc) as tc:
        with (
            tc.tile_pool(name="wpool", bufs=1) as wp,
            tc.tile_pool(name="work", bufs=3) as wk,
            tc.tile_pool(name="gpool", bufs=2) as gp,
        ):
            # constants
            ident = wp.tile([P, P], bf16, tag="ident")
            make_identity(nc, ident[:])
            ident_f = wp.tile([P, P], f32, tag="ident_f")
            make_identity(nc, ident_f[:])
            iota_i = wp.tile([P, P], i32, tag="iota_i")
            nc.gpsimd.iota(iota_i[:], pattern=[[1, P]], base=0, channel_multiplier=0)
            iota_row = wp.tile([P, P], bf16, tag="iota_row")
            nc.vector.tensor_copy(iota_row[:], iota_i[:])
            zero_c = wp.tile([P, 1], bf16, tag="zero_c")
            nc.vector.memset(zero_c[:], 0.0)

            sb = {}

            def load_w(name, rows, cols, dt=bf16):
                t = wp.tile([P, rows * cols], dt, tag=f"sb_{name}")
                nc.sync.dma_start(
                    out=t[:].rearrange("p (t m) -> p t m", t=rows),
                    in_=W[name][:, :].rearrange("(t p) m -> p t m", p=P))
                sb[name] = t

            def load_col(name, rows):  # [rows*128, 1] f32 -> [128, rows]
                t = wp.tile([P, rows], f32, tag=f"sb_{name}")
                nc.sync.dma_start(
                    out=t[:].rearrange("p (a o) -> p a o", a=rows),
                    in_=W[name][:, :].rearrange("(a p) o -> p a o", p=P))
                sb[name] = t

            def load_bcast(name, wd):  # [1, wd] -> [128, wd]
                t = wp.tile([P, wd], bf16, tag=f"sb_{name}")
                nc.sync.dma_start(out=t[:], in_=W[name][0:1, :].to_broadcast([P, wd]))
                sb[name] = t

            for m in ("p", "t"):
                load_w(f"w2_{m}", 1, IN_DIM)
                load_w(f"wl2_{m}", 4, HID)
                load_w(f"wr2_{m}", 4, HID)
                load_col(f"b1_{m}", 1)
                load_col(f"b2_{m}", 2)
            load_w("wl1", 2, D1)
            load_w("wr1", 2, D1)
            load_w("w1e", 2, 4 * HID, dt=f32)
            load_w("w2e", 4, IN_DIM, dt=f32)
            load_col("b2e", 2)
            for nm, ncol in (("att1", 8), ("att2", 2)):
                t = wp.tile([P, ncol], bf16, tag=f"sb_{nm}")
                nc.sync.dma_start(out=t[:], in_=W[nm][:, :])
                sb[nm] = t
            t = wp.tile([P, 4], f32, tag="sb_b1e")
            nc.sync.dma_start(out=t[:], in_=W["b1e"][:, :])
            sb["b1e"] = t
            for nm, wd in (("bl1", D1), ("br1", D1), ("gb1", D1),
                           ("bl2", D2), ("br2", D2), ("gb2", D2)):
                load_bcast(nm, wd)

            src_sb = wp.tile([P, NG * GRP * 8], i16, tag="src_sb")
            nc.sync.dma_start(out=src_sb[:], in_=src16[:, :])
            dst_sb = wp.tile([P, NG * GRP * 8], i16, tag="dst_sb")
            nc.sync.dma_start(out=dst_sb[:], in_=dst16[:, :])
            dcol_sb = wp.tile([P, NBLK * MG], bf16, tag="dcol_sb")
            nc.sync.dma_start(out=dcol_sb[:], in_=dcol_in[:, :])

            x1T = {m: [wp.tile([P, NPAD], bf16, tag=f"x1T_{m}{h}", name=f"x1T_{m}{h}") for h in range(2)]
                   for m in ("p", "t")}
            x2T = {m: [wp.tile([P, NPAD], bf16, tag=f"x2T_{m}{k}", name=f"x2T_{m}{k}") for k in range(4)]
                   for m in ("p", "t")}
            hT = {m: [wp.tile([P, NPAD], f32, tag=f"hT_{m}", name=f"hT_{m}")] for m in ("p", "t")}
            for m in ("p", "t"):
                for t_ in x1T[m] + x2T[m] + hT[m]:
                    nc.vector.memset(t_[:], 0.0)
            eT = [wp.tile([P, NLOC], f32, tag=f"eT{i}", name=f"eT{i}") for i in range(4)]

            ntiles = [(0, 512), (512, 1024), (1024, NLOC)]

            # ================ extractors ================
            with (
                tc.tile_pool(name="ps_e1", bufs=1, space="PSUM") as pse1,
                tc.tile_pool(name="ps_e2", bufs=2, space="PSUM") as pse2,
            ):
                for m in ("p", "t"):
                    hps = [pse1.tile([P, 512], f32, tag=f"hps{i}", name=f"hps{i}")
                           for i in range(3)]
                    for k in range(32):
                        w1k = wk.tile([P, HID], f32, tag="w1k")
                        nc.sync.dma_start(out=w1k[:],
                                          in_=W[f"w1_{m}"][k * P:(k + 1) * P, :])
                        for i, (n0, n1) in enumerate(ntiles):
                            nn = n1 - n0
                            ft = wk.tile([P, 512], f32, tag="ft")
                            nc.sync.dma_start(
                                out=ft[:, :nn],
                                in_=featT[m][k * P:(k + 1) * P, n0:n1])
                            nc.tensor.matmul(
                                out=hps[i][:, :nn], lhsT=w1k[:],
                                rhs=ft[:, :nn], start=(k == 0), stop=(k == 31))
                    for i, (n0, n1) in enumerate(ntiles):
                        nn = n1 - n0
                        gl = wk.tile([P, 512], bf16, tag="gl")
                        nc.scalar.activation(gl[:, :nn], hps[i][:, :nn], ActFn.Gelu,
                                             bias=sb[f"b1_{m}"][:, 0:1])
                        for half in range(2):
                            ops = pse2.tile([P, 512], f32, tag="ops")
                            nc.tensor.matmul(
                                out=ops[:, :nn],
                                lhsT=sb[f"w2_{m}"][:, half * P:(half + 1) * P],
                                rhs=gl[:, :nn], start=True, stop=True)
                            nc.vector.tensor_tensor(
                                out=x1T[m][half][:, n0:n1], in0=ops[:, :nn],
                                in1=sb[f"b2_{m}"][:, half:half + 1].to_broadcast([P, nn]),
                                op=AluOp.add)

            # ================ D1: xl1/xr1 + AllGather ================
            with tc.tile_pool(name="ps_d1", bufs=2, space="PSUM") as psd:
                for it in range(NBLK):
                    n0 = it * P
                    nrow = min(P, NLOC - n0)
                    for wname, bname, dest in (("wl1", "bl1", ag1in),
                                               ("wr1", "br1", XR1)):
                        xt = wk.tile([P, D1], bf16, tag="d1x")
                        for half, m in enumerate(("p", "t")):
                            ps = psd.tile([P, 512], f32, tag="d1ps")
                            for k in range(2):
                                nc.tensor.matmul(
                                    out=ps[:],
                                    lhsT=x1T[m][k][:, n0:n0 + P],
                                    rhs=sb[wname][:, k * D1 + half * 512:
                                                  k * D1 + (half + 1) * 512],
                                    start=(k == 0), stop=(k == 1))
                            nc.vector.tensor_tensor(
                                out=xt[:, half * 512:(half + 1) * 512], in0=ps[:],
                                in1=sb[bname][:, half * 512:(half + 1) * 512],
                                op=AluOp.add)
                        nc.sync.dma_start(out=dest[n0:n0 + nrow, :], in_=xt[:nrow, :])
            nc.gpsimd.collective_compute(
                "AllGather", AluOp.bypass, replica_groups=[list(range(NCORES))],
                ins=[ag1in[:, :].opt()], outs=[XL1[:, :].opt()])

            # ================ G1: gat1 edge stage ================
            _edge_stage(nc, tc, wk, gp, ident, ident_f, iota_row, zero_c,
                        XL=XL1, XR=XR1, src_sb=src_sb, dst_sb=dst_sb,
                        dcol_sb=dcol_sb, D=D1, NH=8, MG=MG, NGPB=NGPB,
                        att=sb["att1"], gbias=sb["gb1"], xT=x2T, nk=4,
                        edge_blocks=edge_blocks, stage="g1")

            # ================ D2: xl2/xr2 + AllGather ================
            with tc.tile_pool(name="ps_d2", bufs=2, space="PSUM") as psd:
                for it in range(NBLK):
                    n0 = it * P
                    nrow = min(P, NLOC - n0)
                    for wkey, bname, dest in (("wl2", "bl2", ag2in),
                                              ("wr2", "br2", XR2)):
                        xt = wk.tile([P, D2], bf16, tag="d2x")
                        ps = psd.tile([P, D2], f32, tag="d2ps")
                        first = True
                        for mi, m in enumerate(("p", "t")):
                            for k in range(4):
                                nc.tensor.matmul(
                                    out=ps[:, mi * HID:(mi + 1) * HID],
                                    lhsT=x2T[m][k][:, n0:n0 + P],
                                    rhs=sb[f"{wkey}_{m}"][:, k * HID:(k + 1) * HID],
                                    start=first, stop=(mi == 1 and k == 3))
                                first = False
                        nc.vector.tensor_tensor(
                            out=xt[:], in0=ps[:], in1=sb[bname][:, :], op=AluOp.add)
                        nc.sync.dma_start(out=dest[n0:n0 + nrow, :], in_=xt[:nrow, :])
            nc.gpsimd.collective_compute(
                "AllGather", AluOp.bypass, replica_groups=[list(range(NCORES))],
                ins=[ag2in[:, :].opt()], outs=[XL2[:, :].opt()])

            # ================ G2: gat2 edge stage ================
            _edge_stage(nc, tc, wk, gp, ident, ident_f, iota_row, zero_c,
                        XL=XL2, XR=XR2, src_sb=src_sb, dst_sb=dst_sb,
                        dcol_sb=dcol_sb, D=D2, NH=2, MG=MG, NGPB=NGPB,
                        att=sb["att2"], gbias=sb["gb2"], xT=hT, nk=1,
                        edge_blocks=edge_blocks, stage="g2", out_dt=f32)

            # ================ enhancer ================
            with tc.tile_pool(name="ps_enh", bufs=2, space="PSUM") as pse:
                for (n0, n1) in ntiles:
                    nn = n1 - n0
                    for ms in range(4):
                        ps = pse.tile([P, 512], f32, tag="eps")
                        for k, m in enumerate(("p", "t")):
                            nc.tensor.matmul(
                                out=ps[:, :nn],
                                lhsT=sb["w1e"][:, k * 512 + ms * P:
                                               k * 512 + (ms + 1) * P],
                                rhs=hT[m][0][:, n0:n1], start=(k == 0), stop=(k == 1))
                        nc.scalar.activation(eT[ms][:, n0:n1], ps[:, :nn], ActFn.Gelu,
                                             bias=sb["b1e"][:, ms:ms + 1])
                for (n0, n1) in ntiles:
                    nn = n1 - n0
                    for half in range(2):
                        ps = pse.tile([P, 512], f32, tag="ops2")
                        for k in range(4):
                            nc.tensor.matmul(
                                out=ps[:, :nn],
                                lhsT=sb["w2e"][:, k * IN_DIM + half * P:
                                               k * IN_DIM + (half + 1) * P],
                                rhs=eT[k][:, n0:n1], start=(k == 0), stop=(k == 3))
                        ot = wk.tile([P, 512], f32, tag="otile")
                        nc.vector.tensor_tensor(
                            out=ot[:, :nn], in0=ps[:, :nn],
                            in1=sb["b2e"][:, half:half + 1].to_broadcast([P, nn]),
                            op=AluOp.add)
                        nc.sync.dma_start(out=outT[half * P:(half + 1) * P, n0:n1],
                                          in_=ot[:, :nn])

    nc.compile()
    return nc


def _edge_stage(nc, tc, wk, gp, ident, ident_f, iota_row, zero_c, *,
                XL, XR, src_sb, dst_sb, dcol_sb, D, NH, MG, NGPB,
                att, gbias, xT, nk, edge_blocks, stage, out_dt=bf16):
    """GATv2 edge stage.  Writes feature-major elu(out+bias)+1 into the
    persistent tiles xT[m][k][:, b*128:(b+1)*128]."""
    Dm = D // 2
    NHm = NH // 2
    with (
        tc.tile_pool(name=f"psV_{stage}", bufs=2, space="PSUM") as psv,
        tc.tile_pool(name=f"psL_{stage}", bufs=1, space="PSUM") as psl,
        tc.tile_pool(name=f"psS_{stage}", bufs=1, space="PSUM") as pss,
    ):
        for b in range(edge_blocks):
            S0 = pss.tile([P, min(D, 512)], f32, tag="S0", name="S0")
            S1 = pss.tile([P, 512], f32, tag="S1", name="S1") if D > 512 else None
            DEN = pss.tile([P, NH], f32, tag="DEN")
            for gi in range(NGPB):
                g_ix = b * NGPB + gi
                i0 = g_ix * GRP * 8
                gG = gp.tile([P, GRP, D], bf16, tag="gG")
                nc.gpsimd.dma_gather(
                    out_ap=gG[:], in_ap=XL[:, :],
                    idxs_ap=src_sb[:, i0:i0 + GRP * 8],
                    num_idxs=GRP * P, num_idxs_reg=GRP * P, elem_size=D)
                gR = gp.tile([P, GRP, D], bf16, tag="gR")
                nc.gpsimd.dma_gather(
                    out_ap=gR[:], in_ap=XR[:, :],
                    idxs_ap=dst_sb[:, i0:i0 + GRP * 8],
                    num_idxs=GRP * P, num_idxs_reg=GRP * P, elem_size=D)
                for c in range(GRP):
                    cidx = gi * GRP + c
                    cg = b * MG + cidx
                    first = cidx == 0
                    last = cidx == MG - 1
                    U = wk.tile([P, D], bf16, tag="U")
                    nc.vector.tensor_tensor(out=U[:], in0=gG[:, c, :],
                                            in1=gR[:, c, :], op=AluOp.add)
                    U2 = wk.tile([P, D], bf16, tag="U2")
                    nc.scalar.mul(U2[:], U[:], 0.2)
                    V = wk.tile([P, D], bf16, tag="V")
                    nc.vector.tensor_tensor(out=V[:], in0=U[:], in1=U2[:],
                                            op=AluOp.max)
                    VT = psv.tile([P, D], bf16, tag="VT")
                    for t in range(NH):
                        nc.tensor.transpose(out=VT[:, t * P:(t + 1) * P],
                                            in_=V[:, t * P:(t + 1) * P],
                                            identity=ident[:])
                    Vs = wk.tile([P, D], bf16, tag="Vs")
                    nc.scalar.copy(Vs[:], VT[:])
                    LG = psl.tile([P, NH], f32, tag="LG")
                    for mh in range(NH):
                        nc.tensor.matmul(out=LG[:, mh:mh + 1],
                                         lhsT=Vs[:, mh * P:(mh + 1) * P],
                                         rhs=att[:, mh:mh + 1],
                                         start=True, stop=True)
                    EX = wk.tile([P, NH], bf16, tag="EX")
                    nc.scalar.activation(EX[:], LG[:], ActFn.Exp)
                    Ae = wk.tile([P, P], bf16, tag="Ae")
                    nc.vector.tensor_tensor(
                        out=Ae[:], in0=iota_row[:],
                        in1=dcol_sb[:, cg:cg + 1].to_broadcast([P, P]),
                        op=AluOp.is_equal)
                    Gw = wk.tile([P, D], bf16, tag="Gw")
                    nc.vector.tensor_tensor(
                        out=_rh(Gw[:], NH), in0=_rh(gG[:, c, :], NH),
                        in1=EX[:].to_broadcast([P, NH, P]), op=AluOp.mult)
                    nc.tensor.matmul(out=S0[:], lhsT=Ae[:],
                                     rhs=Gw[:, :512] if D > 512 else Gw[:],
                                     start=first, stop=last)
                    if S1 is not None:
                        nc.tensor.matmul(out=S1[:], lhsT=Ae[:], rhs=Gw[:, 512:],
                                         start=first, stop=last)
                    nc.tensor.matmul(out=DEN[:], lhsT=Ae[:], rhs=EX[:],
                                     start=first, stop=last)
            # ---- block epilogue: normalize + bias + elu(+1), transpose out
            rcp = wk.tile([P, NH], f32, tag="rcp")
            nc.vector.reciprocal(rcp[:], DEN[:])
            for mi, m in enumerate(("p", "t")):
                Sm = S0 if (S1 is None or mi == 0) else S1
                soff = Dm if (S1 is None and mi == 1) else 0
                tm = wk.tile([P, Dm], out_dt, tag="tm")
                nc.vector.tensor_tensor(
                    out=_rh(tm[:], NHm), in0=_rh(Sm[:, soff:soff + Dm], NHm),
                    in1=rcp[:, mi * NHm:(mi + 1) * NHm].to_broadcast([P, NHm, P]),
                    op=AluOp.mult)
                ub = wk.tile([P, Dm], out_dt, tag="ub")
                nc.vector.tensor_tensor(out=ub[:], in0=tm[:],
                                        in1=gbias[:, mi * Dm:(mi + 1) * Dm],
                                        op=AluOp.add)
                mn = wk.tile([P, Dm], out_dt, tag="mn")
                nc.vector.tensor_tensor(out=mn[:], in0=ub[:],
                                        in1=zero_c[:].to_broadcast([P, Dm]),
                                        op=AluOp.min)
                ep = wk.tile([P, Dm], out_dt, tag="ep")
                nc.scalar.activation(ep[:], mn[:], ActFn.Exp)
                mx = wk.tile([P, Dm], out_dt, tag="mx")
                nc.vector.tensor_tensor(out=mx[:], in0=ub[:],
                                        in1=zero_c[:].to_broadcast([P, Dm]),
                                        op=AluOp.max)
                x2b = wk.tile([P, Dm], out_dt, tag="x2b")
                nc.vector.tensor_tensor(out=x2b[:], in0=ep[:], in1=mx[:],
                                        op=AluOp.add)
                for k in range(nk):
                    tp = psv.tile([P, P], out_dt, tag="VT", name="tp")
                    nc.tensor.transpose(out=tp[:], in_=x2b[:, k * P:(k + 1) * P],
                                        identity=ident[:] if out_dt == bf16 else ident_f[:])
                    nc.scalar.copy(xT[m][k][:, b * P:(b + 1) * P], tp[:])


# ---------------------------------------------------------------- entry point

_CACHE = {}


def _get_nc(MG):
    if MG not in _CACHE:
        _CACHE[MG] = _build(MG)
    return _CACHE[MG]


def kernel(packet_feat, time_feat, edge_index, params):
    srcs, dsts, cols, MG = _prep_edges(edge_index)
    g = _prep_weights(params)
    nc = _get_nc(MG)

    pf = np.asarray(packet_feat, np.float32).reshape(N, FEAT)
    tf = np.asarray(time_feat, np.float32).reshape(N, FEAT)

    in_maps = []
    for r in range(NCORES):
        im = dict(g)
        im["featT_p"] = np.ascontiguousarray(
            pf[r * NLOC:(r + 1) * NLOC].T)
        im["featT_t"] = np.ascontiguousarray(
            tf[r * NLOC:(r + 1) * NLOC].T)
        im["src16"] = srcs[r]
        im["dst16"] = dsts[r]
        im["dcol"] = cols[r]
        in_maps.append(im)

    res = run_bass_kernel_spmd(nc, in_maps, core_ids=list(range(NCORES)))
    out = np.concatenate([res.results[r]["outT"].T for r in range(NCORES)], axis=0)
    return out.astype(np.float32)
